# revision 14
# baseline (speedup 1.0000x reference)
"""Local (banded) attention kernel for Trainium2, sharded over 8 NeuronCores.

Sharding: core c handles batch b=c//4 and heads 4*(c%4)..4*(c%4)+3.
Host pre-transposes x and weight slices; device does QKV projection,
banded attention (window 128 -> only tile-diagonal +/-1 blocks), and the
per-core slice of the output projection. Host sums the 4 partial outputs
per batch and adds the output bias.
"""

import ml_dtypes
import numpy as np

import concourse.bass as bass
import concourse.mybir as mybir
from concourse import bacc
from concourse.tile import TileContext
from concourse.bass_utils import run_bass_kernel_spmd
from concourse.masks import make_identity

B, N, E, H, DH, WIN = 2, 2048, 1024, 16, 64, 128
HPC = 4              # heads per core
SL = HPC * DH        # feature slice per core (256)
NT = N // 128        # 16 query/key tiles
F32 = mybir.dt.float32
F32R = mybir.dt.float32r
BF16 = mybir.dt.bfloat16
SCALE = 1.0 / 32.0   # 1/sqrt(E)
AUXW = 264           # aux rows: 0=bv, 1=ones, 2=zeros

_CACHED_NC = None


def _build_nc():
    nc = bacc.Bacc("TRN2", target_bir_lowering=False)

    xT_d = nc.dram_tensor("xT", [E, N], F32R, kind="ExternalInput")
    wqT_d = nc.dram_tensor("wqT", [E, SL], F32R, kind="ExternalInput")
    wkT_d = nc.dram_tensor("wkT", [E, SL], F32R, kind="ExternalInput")
    wvT_d = nc.dram_tensor("wvT", [E, SL], F32R, kind="ExternalInput")
    wpT_d = nc.dram_tensor("wpT", [SL, E], BF16, kind="ExternalInput")
    bq_d = nc.dram_tensor("bq", [SL], F32, kind="ExternalInput")
    bk_d = nc.dram_tensor("bk", [SL], F32, kind="ExternalInput")
    aux_d = nc.dram_tensor("aux", [3, AUXW], F32R, kind="ExternalInput")
    y_d = nc.dram_tensor("y", [N, E], F32, kind="ExternalOutput")

    KO = E // 128  # 8 contraction tiles

    with TileContext(nc) as tc:
        with (
            tc.tile_pool(name="const", bufs=1) as const,
            tc.tile_pool(name="persist", bufs=1) as persist,
            tc.tile_pool(name="io", bufs=3) as io,
            tc.tile_pool(name="small", bufs=6) as small,
            tc.tile_pool(name="strips", bufs=16) as strip_pool,
            tc.tile_pool(name="ps_mm", bufs=2, space="PSUM") as ps_mm,
            tc.tile_pool(name="ps_e", bufs=2, space="PSUM") as ps_e,
            tc.tile_pool(name="ps_ut", bufs=2, space="PSUM") as ps_ut,
        ):
            # ---- small constants first (cheap DMAs) ----
            t_bv = const.tile([1, SL], F32R, name="t_bv")
            nc.sync.dma_start(t_bv[:], aux_d.ap()[0:1, :SL])
            t_ones = const.tile([1, AUXW], F32R, name="t_ones")
            nc.sync.dma_start(t_ones[:], aux_d.ap()[1:2, :])
            t_zero = const.tile([1, 128], F32R, name="t_zero")
            nc.sync.dma_start(t_zero[:], aux_d.ap()[2:3, :128])
            bv_row = t_bv[:]
            ones_row = t_ones[:, :128]
            zero_row = t_zero[:]
            rhs260 = t_ones[:, :HPC * (DH + 1)]
            bq_col = const.tile([128, 2], F32)
            nc.sync.dma_start(bq_col[:], bq_d.ap().rearrange("(g p) -> p g", p=128))
            bk_col = const.tile([128, 2], F32)
            nc.sync.dma_start(bk_col[:], bk_d.ap().rearrange("(g p) -> p g", p=128))

            ident = const.tile([128, 128], BF16)
            make_identity(nc, ident[:])
            # 3-block band mask [U | ones | L] for the strip of a key tile
            mask3 = const.tile([128, 384], BF16)
            nc.gpsimd.memset(mask3[:], 1.0)
            nc.gpsimd.affine_select(
                out=mask3[:, 0:128], in_=mask3[:, 0:128],
                compare_op=mybir.AluOpType.is_ge, fill=0.0, base=0,
                pattern=[[1, 128]], channel_multiplier=-1)  # keep c >= p
            nc.gpsimd.affine_select(
                out=mask3[:, 256:384], in_=mask3[:, 256:384],
                compare_op=mybir.AluOpType.is_ge, fill=0.0, base=0,
                pattern=[[-1, 128]], channel_multiplier=1)  # keep c <= p

            # ---- weights before x so compute can start early ----
            wq_sb = persist.tile([128, KO, SL], F32R)
            nc.sync.dma_start(wq_sb[:], wqT_d.ap().rearrange("(ko p) m -> p ko m", p=128))
            wk_sb = persist.tile([128, KO, SL], F32R)
            wv_sb = persist.tile([128, KO, SL], F32R)
            wp_sb = persist.tile([128, 2, E], BF16)

            xT_sb = persist.tile([128, KO, N], F32R)
            xT_ap = xT_d.ap().rearrange("(ko p) n -> p ko n", p=128)
            for c8 in range(8):
                s = slice(c8 * (N // 8), (c8 + 1) * (N // 8))
                nc.sync.dma_start(xT_sb[:, :, s], xT_ap[:, :, s])
                if c8 == 1:
                    nc.sync.dma_start(
                        wk_sb[:], wkT_d.ap().rearrange("(ko p) m -> p ko m", p=128))
                    nc.sync.dma_start(
                        wv_sb[:], wvT_d.ap().rearrange("(ko p) m -> p ko m", p=128))
                if c8 == 3:
                    nc.sync.dma_start(
                        wp_sb[:], wpT_d.ap().rearrange("(g p) f -> p g f", p=128))
            NCH = 4
            CW = N // NCH  # 512

            # ---- projection outputs ----
            qT = [persist.tile([128, N], F32R, name=f"qT{g}", tag=f"qT{g}")
                  for g in range(2)]
            kT = [persist.tile([128, N], F32R, name=f"kT{g}", tag=f"kT{g}")
                  for g in range(2)]
            vaug = persist.tile([128, NT, HPC, DH + 1], BF16)
            nc.gpsimd.memset(vaug[:, :, :, DH], 1.0)
            attT = [persist.tile([128, N], BF16, name=f"attT{g}", tag=f"attT{g}")
                    for g in range(2)]

            # ---- phase 2: QKV, interleaved per x-chunk so DMA overlaps ----
            for ch in range(NCH):
                cs = slice(ch * CW, (ch + 1) * CW)
                for w_sb, out_t, b_col in ((wq_sb, qT, bq_col), (wk_sb, kT, bk_col)):
                    for g in range(2):
                        ps = ps_mm.tile([128, 512], F32, tag="mm", name="ps_qk")
                        for kt in range(KO):
                            nc.tensor.matmul(
                                ps[:],
                                lhsT=w_sb[:, kt, g * 128:(g + 1) * 128],
                                rhs=xT_sb[:, kt, cs],
                                start=(kt == 0), stop=(kt == KO - 1))
                        nc.scalar.activation(
                            out_t[g][:, cs], ps[:],
                            mybir.ActivationFunctionType.Identity,
                            bias=b_col[:, g:g + 1])
                for nt in range(ch * NCH, (ch + 1) * NCH):
                    ps = ps_mm.tile([128, 512], F32, tag="mm", name="ps_v")
                    psv = ps[:, :SL]
                    rs = slice(nt * 128, (nt + 1) * 128)
                    for kt in range(KO):
                        nc.tensor.matmul(
                            psv, lhsT=xT_sb[:, kt, rs], rhs=wv_sb[:, kt, :],
                            start=(kt == 0), stop=False)
                    nc.tensor.matmul(
                        psv, lhsT=ones_row, rhs=bv_row,
                        start=False, stop=True)
                    nc.vector.tensor_copy(
                        vaug[:, nt, :, :DH],
                        psv.rearrange("p (h d) -> p h d", d=DH))

            # ---- phase 3+4: banded attention, fused projection + store ----
            strips = {}

            def emit_strip_pair(gp, kj):
                lo, hi = max(0, kj - 1), min(NT - 1, kj + 1)
                w = (hi - lo + 1) * 128
                moff = 0 if lo == kj - 1 else 128
                pe2 = ps_e.tile([128, 2, 512], F32, tag="pe", name="pe")
                for hh in range(2):
                    po = hh * 64
                    nc.tensor.matmul(
                        pe2[:, hh, :w],
                        lhsT=kT[gp][po:po + 64, kj * 128:(kj + 1) * 128],
                        rhs=qT[gp][po:po + 64, lo * 128:(hi + 1) * 128],
                        start=True, stop=True)
                st2 = strip_pool.tile([128, 2, 384], BF16, tag="strip", name="st")
                nc.scalar.activation(
                    st2[:, :, :w], pe2[:, :, :w],
                    mybir.ActivationFunctionType.Exp, scale=SCALE)
                nc.vector.tensor_mul(
                    st2[:, :, :w], st2[:, :, :w],
                    mask3[:, None, moff:moff + w].to_broadcast((128, 2, w)))
                strips[(2 * gp, kj)] = (st2[:, 0, :], lo)
                strips[(2 * gp + 1, kj)] = (st2[:, 1, :], lo)

            def process_tile(t):
                ts_ = slice(t * 128, (t + 1) * 128)
                ks = [k for k in (t - 1, t, t + 1) if 0 <= k < NT]
                pu = ps_ut.tile([128, HPC, DH + 1], F32, tag="ut", name="pu")
                # zero-fill the whole bank so the 12 AV matmuls accumulate
                # order-independently (has_written set everywhere once)
                nc.tensor.matmul(
                    pu[:], lhsT=zero_row, rhs=rhs260,
                    start=True, stop=False, skip_group_check=True)
                for h in range(HPC):
                    for i, k2 in enumerate(ks):
                        st, lo2 = strips[(h, k2)]
                        col = (t - lo2) * 128
                        nc.tensor.matmul(
                            pu[:, h, :], lhsT=st[:, col:col + 128],
                            rhs=vaug[:, k2, h, :],
                            start=False,
                            stop=(h == HPC - 1 and i == len(ks) - 1),
                            skip_group_check=True)
                rec = small.tile([128, HPC], F32, tag="rec", name="rec")
                nc.vector.reciprocal(rec[:], pu[:, :, DH])
                ao = small.tile([128, HPC, DH], BF16, tag="ao", name="ao")
                for h in range(HPC):
                    nc.vector.tensor_scalar_mul(
                        ao[:, h, :], pu[:, h, :DH], rec[:, h:h + 1])
                for g in range(2):
                    pt = ps_ut.tile([128, 128], BF16, tag="ut", name="pt")
                    nc.tensor.transpose(
                        pt[:], ao[:, 2 * g:2 * g + 2, :], ident[:])
                    if g == 0:
                        nc.scalar.activation(
                            attT[g][:, ts_], pt[:],
                            mybir.ActivationFunctionType.Copy)
                    else:
                        nc.vector.tensor_copy(attT[g][:, ts_], pt[:])
                # fused output projection for this token tile
                y_sb = io.tile([128, E], F32, tag="y", name="y_sb")
                for fc in range(2):
                    ps = ps_mm.tile([128, 512], F32, tag="mm", name="ps_y")
                    fs = slice(fc * 512, (fc + 1) * 512)
                    for g in range(2):
                        nc.tensor.matmul(
                            ps[:],
                            lhsT=attT[g][:, ts_],
                            rhs=wp_sb[:, g, fs],
                            start=(g == 0), stop=(g == 1))
                    if fc == 0:
                        nc.scalar.activation(
                            y_sb[:, fs], ps[:],
                            mybir.ActivationFunctionType.Copy)
                    else:
                        nc.vector.tensor_copy(y_sb[:, fs], ps[:])
                nc.sync.dma_start(y_d[ts_, :], y_sb[:])

            for kj in range(NT):
                for gp in range(2):
                    emit_strip_pair(gp, kj)
                if kj >= 1:
                    process_tile(kj - 1)
            process_tile(NT - 1)

    nc.compile()
    return nc


def _get_nc():
    global _CACHED_NC
    if _CACHED_NC is None:
        _CACHED_NC = _build_nc()
    return _CACHED_NC


def kernel(x, Wq, bq, Wk, bk, Wv, bv, Wp, bp):
    nc = _get_nc()
    x = np.asarray(x, np.float32)
    xTs = [np.ascontiguousarray(x[b].T) for b in range(B)]
    in_maps = []
    for c in range(8):
        b, gq = c // 4, c % 4
        sl = slice(SL * gq, SL * (gq + 1))
        aux = np.zeros((3, AUXW), np.float32)
        aux[0, :SL] = np.asarray(bv, np.float32)[sl]
        aux[1, :] = 1.0
        in_maps.append({
            "xT": xTs[b],
            "wqT": np.ascontiguousarray(np.asarray(Wq, np.float32)[sl].T),
            "wkT": np.ascontiguousarray(np.asarray(Wk, np.float32)[sl].T),
            "wvT": np.ascontiguousarray(np.asarray(Wv, np.float32)[sl].T),
            "wpT": np.ascontiguousarray(
                np.asarray(Wp, np.float32)[:, sl].T).astype(ml_dtypes.bfloat16),
            "bq": np.ascontiguousarray(np.asarray(bq, np.float32)[sl]),
            "bk": np.ascontiguousarray(np.asarray(bk, np.float32)[sl]),
            "aux": aux,
        })
    res = run_bass_kernel_spmd(nc, in_maps, core_ids=list(range(8)))
    ys = [res.results[c]["y"] for c in range(8)]
    bp = np.asarray(bp, np.float32)
    y = np.stack([
        ys[0] + ys[1] + ys[2] + ys[3],
        ys[4] + ys[5] + ys[6] + ys[7],
    ]).astype(np.float32) + bp[None, None, :]
    return y.astype(np.float32)


# revision 15
# speedup vs baseline: 1.1322x; 1.1322x over previous
"""Local (banded) attention kernel for Trainium2, sharded over 8 NeuronCores.

Sharding: core c handles batch b=c//4 and heads 4*(c%4)..4*(c%4)+3.
Host pre-transposes x and weight slices; device does QKV projection,
banded attention (window 128 -> only tile-diagonal +/-1 blocks), and the
per-core slice of the output projection. Host sums the 4 partial outputs
per batch and adds the output bias.
"""

import ml_dtypes
import numpy as np

import concourse.bass as bass
import concourse.mybir as mybir
from concourse import bacc
from concourse.tile import TileContext
from concourse.bass_utils import run_bass_kernel_spmd
from concourse.masks import make_identity

B, N, E, H, DH, WIN = 2, 2048, 1024, 16, 64, 128
HPC = 4              # heads per core
SL = HPC * DH        # feature slice per core (256)
NT = N // 128        # 16 query/key tiles
F32 = mybir.dt.float32
F32R = mybir.dt.float32r
BF16 = mybir.dt.bfloat16
SCALE = 1.0 / 32.0   # 1/sqrt(E)
AUXW = 264           # aux rows: 0=bv, 1=ones, 2=zeros

_CACHED_NC = None


def _build_nc():
    nc = bacc.Bacc("TRN2", target_bir_lowering=False)

    xT_d = nc.dram_tensor("xT", [E, N], F32R, kind="ExternalInput")
    wqT_d = nc.dram_tensor("wqT", [E, SL], F32R, kind="ExternalInput")
    wkT_d = nc.dram_tensor("wkT", [E, SL], F32R, kind="ExternalInput")
    wvT_d = nc.dram_tensor("wvT", [E, SL], F32R, kind="ExternalInput")
    wpT_d = nc.dram_tensor("wpT", [SL, E], BF16, kind="ExternalInput")
    bq_d = nc.dram_tensor("bq", [SL], F32, kind="ExternalInput")
    bk_d = nc.dram_tensor("bk", [SL], F32, kind="ExternalInput")
    aux_d = nc.dram_tensor("aux", [3, AUXW], F32R, kind="ExternalInput")
    y_d = nc.dram_tensor("y", [N, E], F32, kind="ExternalOutput")

    KO = E // 128  # 8 contraction tiles

    with TileContext(nc) as tc:
        with (
            tc.tile_pool(name="const", bufs=1) as const,
            tc.tile_pool(name="persist", bufs=1) as persist,
            tc.tile_pool(name="io", bufs=3) as io,
            tc.tile_pool(name="small", bufs=6) as small,
            tc.tile_pool(name="strips", bufs=16) as strip_pool,
            tc.tile_pool(name="ps_mm", bufs=2, space="PSUM") as ps_mm,
            tc.tile_pool(name="ps_e", bufs=2, space="PSUM") as ps_e,
            tc.tile_pool(name="ps_u", bufs=2, space="PSUM") as ps_u,
            tc.tile_pool(name="ps_t", bufs=2, space="PSUM") as ps_t,
        ):
            # ---- small constants first (cheap DMAs) ----
            t_bv = const.tile([1, SL], F32R, name="t_bv")
            nc.sync.dma_start(t_bv[:], aux_d.ap()[0:1, :SL])
            t_ones = const.tile([1, AUXW], F32R, name="t_ones")
            nc.sync.dma_start(t_ones[:], aux_d.ap()[1:2, :])
            t_zero = const.tile([1, 128], F32R, name="t_zero")
            nc.sync.dma_start(t_zero[:], aux_d.ap()[2:3, :128])
            bv_row = t_bv[:]
            ones_row = t_ones[:, :128]
            zero_row = t_zero[:]
            rhs260 = t_ones[:, :HPC * (DH + 1)]
            bq_col = const.tile([128, 2], F32)
            nc.sync.dma_start(bq_col[:], bq_d.ap().rearrange("(g p) -> p g", p=128))
            bk_col = const.tile([128, 2], F32)
            nc.sync.dma_start(bk_col[:], bk_d.ap().rearrange("(g p) -> p g", p=128))

            ident = const.tile([128, 128], BF16)
            make_identity(nc, ident[:])
            # 3-block band mask [U | ones | L] for the strip of a key tile
            mask3 = const.tile([128, 384], BF16)
            nc.gpsimd.memset(mask3[:], 1.0)
            nc.gpsimd.affine_select(
                out=mask3[:, 0:128], in_=mask3[:, 0:128],
                compare_op=mybir.AluOpType.is_ge, fill=0.0, base=0,
                pattern=[[1, 128]], channel_multiplier=-1)  # keep c >= p
            nc.gpsimd.affine_select(
                out=mask3[:, 256:384], in_=mask3[:, 256:384],
                compare_op=mybir.AluOpType.is_ge, fill=0.0, base=0,
                pattern=[[-1, 128]], channel_multiplier=1)  # keep c <= p

            # ---- weights before x so compute can start early ----
            wq_sb = persist.tile([128, KO, SL], F32R)
            nc.sync.dma_start(wq_sb[:], wqT_d.ap().rearrange("(ko p) m -> p ko m", p=128))
            wk_sb = persist.tile([128, KO, SL], F32R)
            wv_sb = persist.tile([128, KO, SL], F32R)
            wp_sb = persist.tile([128, 2, E], BF16)

            xT_sb = persist.tile([128, KO, N], F32R)
            xT_ap = xT_d.ap().rearrange("(ko p) n -> p ko n", p=128)
            for c8 in range(8):
                s = slice(c8 * (N // 8), (c8 + 1) * (N // 8))
                nc.sync.dma_start(xT_sb[:, :, s], xT_ap[:, :, s])
                if c8 == 1:
                    nc.sync.dma_start(
                        wk_sb[:], wkT_d.ap().rearrange("(ko p) m -> p ko m", p=128))
                    nc.sync.dma_start(
                        wv_sb[:], wvT_d.ap().rearrange("(ko p) m -> p ko m", p=128))
                if c8 == 3:
                    nc.sync.dma_start(
                        wp_sb[:], wpT_d.ap().rearrange("(g p) f -> p g f", p=128))
            NCH = 4
            CW = N // NCH  # 512

            # ---- projection outputs ----
            qT = [persist.tile([128, N], F32R, name=f"qT{g}", tag=f"qT{g}")
                  for g in range(2)]
            kT = [persist.tile([128, N], F32R, name=f"kT{g}", tag=f"kT{g}")
                  for g in range(2)]
            vaug = persist.tile([128, NT, HPC, DH + 1], BF16)
            nc.gpsimd.memset(vaug[:, :, :, DH], 1.0)
            attT = [persist.tile([128, N], BF16, name=f"attT{g}", tag=f"attT{g}")
                    for g in range(2)]

            # ---- phase 2: QKV, interleaved per x-chunk so DMA overlaps ----
            for ch in range(NCH):
                cs = slice(ch * CW, (ch + 1) * CW)
                for w_sb, out_t, b_col in ((wq_sb, qT, bq_col), (wk_sb, kT, bk_col)):
                    for g in range(2):
                        ps = ps_mm.tile([128, 512], F32, tag="mm", name="ps_qk")
                        for kt in range(KO):
                            nc.tensor.matmul(
                                ps[:],
                                lhsT=w_sb[:, kt, g * 128:(g + 1) * 128],
                                rhs=xT_sb[:, kt, cs],
                                start=(kt == 0), stop=(kt == KO - 1))
                        nc.scalar.activation(
                            out_t[g][:, cs], ps[:],
                            mybir.ActivationFunctionType.Identity,
                            bias=b_col[:, g:g + 1])
                for nt in range(ch * NCH, (ch + 1) * NCH):
                    ps = ps_mm.tile([128, 512], F32, tag="mm", name="ps_v")
                    psv = ps[:, :SL]
                    rs = slice(nt * 128, (nt + 1) * 128)
                    for kt in range(KO):
                        nc.tensor.matmul(
                            psv, lhsT=xT_sb[:, kt, rs], rhs=wv_sb[:, kt, :],
                            start=(kt == 0), stop=False)
                    nc.tensor.matmul(
                        psv, lhsT=ones_row, rhs=bv_row,
                        start=False, stop=True)
                    nc.vector.tensor_copy(
                        vaug[:, nt, :, :DH],
                        psv.rearrange("p (h d) -> p h d", d=DH))

            # ---- phase 3+4: banded attention, fused projection + store ----
            strips = {}

            def emit_strip(h, kj):
                g, po = h // 2, (h % 2) * 64
                qh = qT[g][po:po + 64, :]
                kh = kT[g][po:po + 64, :]
                lo, hi = max(0, kj - 1), min(NT - 1, kj + 1)
                w = (hi - lo + 1) * 128
                moff = 0 if lo == kj - 1 else 128
                pe = ps_e.tile([128, 384], F32, tag="pe", name="pe")
                nc.tensor.matmul(
                    pe[:, :w],
                    lhsT=kh[:, kj * 128:(kj + 1) * 128],
                    rhs=qh[:, lo * 128:(hi + 1) * 128],
                    start=True, stop=True)
                st = strip_pool.tile([128, 384], BF16, tag="strip", name="st")
                nc.scalar.activation(
                    st[:, :w], pe[:, :w],
                    mybir.ActivationFunctionType.Exp, scale=SCALE)
                nc.vector.tensor_mul(
                    st[:, :w], st[:, :w], mask3[:, moff:moff + w])
                strips[(h, kj)] = (st, lo)

            def process_tile(t):
                ts_ = slice(t * 128, (t + 1) * 128)
                ks = [k for k in (t - 1, t, t + 1) if 0 <= k < NT]
                pu = ps_u.tile([128, HPC, DH + 1], F32, tag="pu", name="pu")
                # zero-fill the whole bank so the 12 AV matmuls accumulate
                # order-independently (has_written set everywhere once)
                nc.tensor.matmul(
                    pu[:], lhsT=zero_row, rhs=rhs260,
                    start=True, stop=False, skip_group_check=True)
                for h in range(HPC):
                    for i, k2 in enumerate(ks):
                        st, lo2 = strips[(h, k2)]
                        col = (t - lo2) * 128
                        nc.tensor.matmul(
                            pu[:, h, :], lhsT=st[:, col:col + 128],
                            rhs=vaug[:, k2, h, :],
                            start=False,
                            stop=(h == HPC - 1 and i == len(ks) - 1),
                            skip_group_check=True)
                rec = small.tile([128, HPC], F32, tag="rec", name="rec")
                nc.vector.reciprocal(rec[:], pu[:, :, DH])
                ao = small.tile([128, HPC, DH], BF16, tag="ao", name="ao")
                for h in range(HPC):
                    nc.vector.tensor_scalar_mul(
                        ao[:, h, :], pu[:, h, :DH], rec[:, h:h + 1])
                for g in range(2):
                    pt = ps_t.tile([128, 128], BF16, tag="pt", name="pt")
                    nc.tensor.transpose(
                        pt[:], ao[:, 2 * g:2 * g + 2, :], ident[:])
                    if g == 0:
                        nc.scalar.activation(
                            attT[g][:, ts_], pt[:],
                            mybir.ActivationFunctionType.Copy)
                    else:
                        nc.vector.tensor_copy(attT[g][:, ts_], pt[:])
                # fused output projection for this token tile
                y_sb = io.tile([128, E], F32, tag="y", name="y_sb")
                for fc in range(2):
                    ps = ps_mm.tile([128, 512], F32, tag="mm", name="ps_y")
                    fs = slice(fc * 512, (fc + 1) * 512)
                    for g in range(2):
                        nc.tensor.matmul(
                            ps[:],
                            lhsT=attT[g][:, ts_],
                            rhs=wp_sb[:, g, fs],
                            start=(g == 0), stop=(g == 1))
                    if fc == 0:
                        nc.scalar.activation(
                            y_sb[:, fs], ps[:],
                            mybir.ActivationFunctionType.Copy)
                    else:
                        nc.vector.tensor_copy(y_sb[:, fs], ps[:])
                nc.sync.dma_start(y_d[ts_, :], y_sb[:])

            for kj in range(NT):
                for h in range(HPC):
                    emit_strip(h, kj)
                if kj >= 1:
                    process_tile(kj - 1)
            process_tile(NT - 1)

    nc.compile()
    return nc


def _get_nc():
    global _CACHED_NC
    if _CACHED_NC is None:
        _CACHED_NC = _build_nc()
    return _CACHED_NC


def kernel(x, Wq, bq, Wk, bk, Wv, bv, Wp, bp):
    nc = _get_nc()
    x = np.asarray(x, np.float32)
    xTs = [np.ascontiguousarray(x[b].T) for b in range(B)]
    in_maps = []
    for c in range(8):
        b, gq = c // 4, c % 4
        sl = slice(SL * gq, SL * (gq + 1))
        aux = np.zeros((3, AUXW), np.float32)
        aux[0, :SL] = np.asarray(bv, np.float32)[sl]
        aux[1, :] = 1.0
        in_maps.append({
            "xT": xTs[b],
            "wqT": np.ascontiguousarray(np.asarray(Wq, np.float32)[sl].T),
            "wkT": np.ascontiguousarray(np.asarray(Wk, np.float32)[sl].T),
            "wvT": np.ascontiguousarray(np.asarray(Wv, np.float32)[sl].T),
            "wpT": np.ascontiguousarray(
                np.asarray(Wp, np.float32)[:, sl].T).astype(ml_dtypes.bfloat16),
            "bq": np.ascontiguousarray(np.asarray(bq, np.float32)[sl]),
            "bk": np.ascontiguousarray(np.asarray(bk, np.float32)[sl]),
            "aux": aux,
        })
    res = run_bass_kernel_spmd(nc, in_maps, core_ids=list(range(8)))
    ys = [res.results[c]["y"] for c in range(8)]
    bp = np.asarray(bp, np.float32)
    y = np.stack([
        ys[0] + ys[1] + ys[2] + ys[3],
        ys[4] + ys[5] + ys[6] + ys[7],
    ]).astype(np.float32) + bp[None, None, :]
    return y.astype(np.float32)
